# revision 9
# baseline (speedup 1.0000x reference)
"""CharAttention TRN2 kernel v4: 8-core data-parallel, ragged-packed,
round-pipelined, stall-free scheduling.

Only the LAST valid character's attention output is consumed by the
reference, so per word we compute
  q      = x[last] @ Wq                 (one query per word)
  K, V   = x @ Wk, x @ Wv               (valid positions only)
  scores = q . K / sqrt(hd) + pen, pen = -1e30 at j >= len
  o      = softmax(scores) @ V
  out    = (o + pos) @ Wp

Ragged packing: words are globally sorted by length (desc) and dealt
round-robin to the 8 cores, so position k on every core has nearly the
same length. A DP picks "rounds" of W_r words padded to a common
per-round length L_r with W_r*L_r <= 512 tokens, cutting the K/V
projection work from 4096 tokens/core to ~2400.

Round-outer pipeline: per round, the K projection accumulates 16-head
scores directly in PSUM (one matmul per f-tile against a head-summing
0/1 matrix, plus one 1-row matmul adding the -1e30 padding penalty),
softmax runs on ACT/DVE while the V projections keep the PE busy (the
head-broadcast matmuls of softmax(p) are skewed 2 f-tiles behind their
V projection so the PE never waits on the softmax chain — any PE gap
>3.4us re-throttles the clock to 1.2GHz). x arrives per (e-tile,
round) segment; K/V/P weights stay resident in SBUF (P prefetched
mid-pipeline); the output projection interleaves per-block. All
matmuls are float32r (1 cycle/row at N>=256); Q is bf16 (f32r is 4x
penalized at N=128).
"""
import os
import numpy as np

B, W, CC, C = 4, 256, 32, 1024
H, HD = 16, 64
NCORES = 8
WPC = (B * W) // NCORES          # 128 words per core
NE = 8                           # e-tiles (contraction over C)
NF = 8                           # f-tiles per 1024-wide projection
RLAM = 48                        # per-round fixed-cost in token units
SKEW = 2                         # pspe lags its psv by this many f-tiles

_cache = {}
LAST_EXEC_NS = None


def _round_shape(Wr, Lk):
    """Effective (L, tokens, pe_cost) for a round of Wr words, max len Lk.

    L >= Lk, total token count even (f32r matmul ISA requires an even
    free size) and >= 256 where possible (f32r is 4x penalized below)."""
    L = max(int(Lk), 1)
    if Wr * L < 256:
        L = min(CC, max(L, -(-256 // Wr)))
    if (Wr * L) % 2 and L < CC:
        L += 1
    T = Wr * L
    pe = T if T >= 256 else 4 * T
    return L, T, pe


def _make_rounds(Lpos):
    """DP partition of the 128 desc-sorted positions into rounds minimizing
    PE cost (padded tokens, <256-wide f32r penalty, per-round overhead)."""
    INF = float("inf")
    f = [INF] * (WPC + 1)
    nxt = [0] * (WPC + 1)
    f[WPC] = 0.0
    for k in range(WPC - 1, -1, -1):
        Lk = max(int(Lpos[k]), 1)
        wmax = min(max(512 // Lk, 1), WPC - k)
        for j in range(k + 1, k + wmax + 1):
            _, T, pe = _round_shape(j - k, Lk)
            if T > 512:
                continue
            c = f[j] + pe + RLAM
            if c < f[k]:
                f[k] = c
                nxt[k] = j
    rounds = []
    k = 0
    while k < WPC:
        j = nxt[k]
        L, _, _ = _round_shape(j - k, Lpos[k])
        rounds.append((j - k, L))
        k = j
    return tuple(rounds)


def _build_nc(rounds):
    import concourse.mybir as mybir
    import concourse.tile as tile
    from concourse import bacc

    f32 = mybir.dt.float32
    f32r = mybir.dt.float32r
    bf16 = mybir.dt.bfloat16
    Exp = mybir.ActivationFunctionType.Exp
    AX = mybir.AxisListType.X

    TP = sum(w * l for w, l in rounds)
    NR = len(rounds)
    woff = np.cumsum([0] + [w for w, _ in rounds])      # word offsets
    toff = np.cumsum([0] + [w * l for w, l in rounds])  # token offsets

    nc = bacc.Bacc("TRN2", target_bir_lowering=False, num_devices=NCORES,
                   debug=False)

    xT = nc.declare_dram_parameter("xT", [NE, 128, TP], f32r, isOutput=False)
    xlT = nc.declare_dram_parameter("xlT", [128, C], bf16, isOutput=False)
    wq_d = nc.declare_dram_parameter("wq_d", [NF, 128, C], bf16, isOutput=False)
    wk_d = nc.declare_dram_parameter("wk_d", [NF, 128, C], f32r, isOutput=False)
    wv_d = nc.declare_dram_parameter("wv_d", [NF, 128, C], f32r, isOutput=False)
    wp_d = nc.declare_dram_parameter("wp_d", [NF, 128, C], f32r, isOutput=False)
    posT = nc.declare_dram_parameter("posT", [128, C], f32, isOutput=False)
    pen_in = nc.declare_dram_parameter("pen_in", [1, TP], bf16, isOutput=False)
    one16_in = nc.declare_dram_parameter("one16_in", [1, 16], bf16,
                                         isOutput=False)
    e16_in = nc.declare_dram_parameter("e16_in", [16, C], f32r, isOutput=False)
    hw2_in = nc.declare_dram_parameter("hw2_in", [128, 32], f32r, isOutput=False)
    out = nc.declare_dram_parameter("out", [128, C], f32, isOutput=True)

    with tile.TileContext(nc) as tc:
        with tc.tile_pool(name="xpool", bufs=2) as xpool, \
             tc.tile_pool(name="wres", bufs=1) as wres, \
             tc.tile_pool(name="wpool", bufs=2) as wpool, \
             tc.tile_pool(name="work", bufs=2) as work, \
             tc.tile_pool(name="small", bufs=1) as small, \
             tc.tile_pool(name="psA", bufs=5, space="PSUM") as psA, \
             tc.tile_pool(name="psS", bufs=1, space="PSUM") as psS:

            # ---- resident loads (Q inputs first so PE starts early) ----
            xlT_sb = small.tile([128, C], bf16)
            nc.sync.dma_start(xlT_sb[:], xlT[:])

            qT_sb = small.tile([128, C], f32)
            s_sb = small.tile([16, WPC], f32)
            oT_sb = small.tile([128, C], f32r)

            # ---- Q projection: qT[f, w] accumulated over e-tiles (bf16) ----
            for i in range(NF):
                wq_t = wpool.tile([128, C], bf16, tag="wq")
                nc.sync.dma_start(wq_t[:], wq_d[i])
                psq = psA.tile([128, 128], f32, tag="mm")
                for t in range(NE):
                    nc.tensor.matmul(
                        psq[:], wq_t[:, t * 128:(t + 1) * 128],
                        xlT_sb[:, t * 128:(t + 1) * 128],
                        start=(t == 0), stop=(t == NE - 1))
                nc.vector.tensor_copy(qT_sb[:, i * 128:(i + 1) * 128], psq[:])

            # x segments for round 0 (scalar queue, in parallel with weights)
            xsegs = {}
            for t in range(NE):
                xs = xpool.tile([128, rounds[0][0] * rounds[0][1]], f32r,
                                tag=f"x{t}", name=f"xs{t}_0")
                nc.scalar.dma_start(xs[:], xT[t][:, 0:int(toff[1])])
                xsegs[(t, 0)] = xs

            # K weights resident (needed all within round 0)
            wk_sb = []
            for i in range(NF):
                wk_t = wres.tile([128, C], f32r, name=f"wk{i}")
                nc.sync.dma_start(wk_t[:], wk_d[i])
                wk_sb.append(wk_t)

            hw2_sb = small.tile([128, 32], f32r)
            nc.sync.dma_start(hw2_sb[:], hw2_in[:])
            one16_sb = small.tile([1, 16], bf16)
            nc.sync.dma_start(one16_sb[:], one16_in[:])
            pen_sb = small.tile([1, TP], bf16)
            nc.sync.dma_start(pen_sb[:], pen_in[:])
            e16_sb = small.tile([16, C], f32r)
            nc.sync.dma_start(e16_sb[:], e16_in[:])

            # V weights resident (needed all within round 0's V phase)
            wv_sb = []
            for i in range(NF):
                wv_t = wres.tile([128, C], f32r, name=f"wv{i}")
                nc.sync.dma_start(wv_t[:], wv_d[i])
                wv_sb.append(wv_t)

            posT_sb = small.tile([128, C], f32)
            nc.sync.dma_start(posT_sb[:], posT[:])

            wp_sb = []

            # ---- round pipeline ----
            for r in range(NR):
                Wr, Lr = rounds[r]
                TRr = Wr * Lr
                tseg = slice(int(toff[r]), int(toff[r + 1]))

                # prefetch x segments for round r+1
                if r + 1 < NR:
                    Wn, Ln = rounds[r + 1]
                    for t in range(NE):
                        xs = xpool.tile([128, Wn * Ln], f32r, tag=f"x{t}",
                                        name=f"xs{t}_{r + 1}")
                        nc.scalar.dma_start(
                            xs[:], xT[t][:, int(toff[r + 1]):int(toff[r + 2])])
                        xsegs[(t, r + 1)] = xs

                # K projection, scores accumulated in PSUM over f-tiles;
                # a final 1-row matmul adds the -1e30 padding penalty
                pss = psS.tile([16, TRr], f32, tag="psS")
                for i in range(NF):
                    psk = psA.tile([128, TRr], f32, tag="mm")
                    for t in range(NE):
                        nc.tensor.matmul(
                            psk[:], wk_sb[i][:, t * 128:(t + 1) * 128],
                            xsegs[(t, r)][:], start=(t == 0),
                            stop=(t == NE - 1))
                    # prod[(2h,64d), (w,j)] = K^T * qT broadcast over j
                    prod = work.tile([128, TRr], f32r, tag="prod")
                    qv = qT_sb[:, i * 128 + int(woff[r]):
                               i * 128 + int(woff[r + 1])]
                    nc.vector.tensor_mul(
                        prod[:].rearrange("p (w j) -> p w j", j=Lr),
                        psk[:].rearrange("p (w j) -> p w j", j=Lr),
                        qv[:, :, None].broadcast_to([128, Wr, Lr]))
                    # scores[h, (w,j)] += Hsum_i.T @ prod  (sums over d)
                    nc.tensor.matmul(
                        pss[:], hw2_sb[:, 14 - 2 * i: 30 - 2 * i],
                        prod[:], start=(i == 0), stop=False)
                nc.tensor.matmul(pss[:], one16_sb[:], pen_sb[:, tseg],
                                 start=False, stop=True)

                # softmax over j straight out of PSUM (padding already -inf)
                probs = work.tile([16, TRr], f32r, tag="probs")
                sv = s_sb[:, int(woff[r]):int(woff[r + 1])]
                nc.scalar.activation(probs[:], pss[:], Exp,
                                     scale=1.0 / float(np.sqrt(HD)))
                nc.vector.reduce_sum(
                    sv, probs[:].rearrange("p (w j) -> p w j", j=Lr), axis=AX)
                nc.vector.reciprocal(sv, sv)
                nc.vector.tensor_mul(
                    probs[:].rearrange("p (w j) -> p w j", j=Lr),
                    probs[:].rearrange("p (w j) -> p w j", j=Lr),
                    sv[:, :, None].broadcast_to([16, Wr, Lr]))

                # V projection; the probs head-broadcast (pspe) for f-tile i
                # is emitted after psv(i+SKEW) so the PE never waits on the
                # softmax chain
                psv_q = []

                def v_tail(i2):
                    psv2 = psv_q[i2]
                    pspe = psA.tile([128, TRr], f32, tag="pe",
                                    name=f"pspe{i2}_{r}", bufs=2)
                    nc.tensor.matmul(
                        pspe[:], e16_sb[:, i2 * 128:(i2 + 1) * 128],
                        probs[:], start=True, stop=True)
                    pexp_sb = work.tile([128, TRr], f32, tag="pexp",
                                        name=f"pexp{i2}_{r}")
                    nc.scalar.copy(pexp_sb[:], pspe[:])
                    prodv = work.tile([128, TRr], f32, tag="prodv",
                                      name=f"prodv{i2}_{r}")
                    nc.vector.tensor_mul(prodv[:], psv2[:], pexp_sb[:])
                    with nc.allow_low_precision(
                            reason="f32r out is bit-identical to f32"):
                        nc.vector.reduce_sum(
                            oT_sb[:, i2 * 128 + int(woff[r]):
                                  i2 * 128 + int(woff[r + 1])],
                            prodv[:].rearrange("p (w j) -> p w j", j=Lr),
                            axis=AX)

                for i in range(NF):
                    psv = psA.tile([128, TRr], f32, tag="mm",
                                   name=f"psv{i}_{r}")
                    for t in range(NE):
                        nc.tensor.matmul(
                            psv[:], wv_sb[i][:, t * 128:(t + 1) * 128],
                            xsegs[(t, r)][:], start=(t == 0),
                            stop=(t == NE - 1))
                    psv_q.append(psv)
                    if i >= SKEW:
                        v_tail(i - SKEW)
                for i2 in range(NF - SKEW, NF):
                    v_tail(i2)

                # prefetch the output-projection weights mid-pipeline
                if r == 1:
                    for i in range(NF):
                        wp_t = wres.tile([128, C], f32r, name=f"wp{i}")
                        nc.sync.dma_start(wp_t[:], wp_d[i])
                        wp_sb.append(wp_t)

            # ---- output projection: out[w, g] = (oT + posT).T @ Wp ----
            for i in range(NF):
                blk = slice(i * 128, (i + 1) * 128)
                nc.vector.tensor_add(oT_sb[:, blk], oT_sb[:, blk],
                                     posT_sb[:, blk])
            out_sb = small.tile([128, C], f32)
            pso = [psA.tile([128, 512], f32, tag="pe", name=f"pso{ch}",
                            bufs=2) for ch in range(2)]
            for i in range(NF):
                for ch in range(2):
                    nc.tensor.matmul(
                        pso[ch][:], oT_sb[:, i * 128:(i + 1) * 128],
                        wp_sb[i][:, ch * 512:(ch + 1) * 512],
                        start=(i == 0), stop=(i == NF - 1))
            for ch in range(2):
                nc.vector.tensor_copy(out_sb[:, ch * 512:(ch + 1) * 512],
                                      pso[ch][:])
                nc.sync.dma_start(out[:, ch * 512:(ch + 1) * 512],
                                  out_sb[:, ch * 512:(ch + 1) * 512])

    nc.finalize()
    return nc


def _tile_lhsT(m):
    """[C, n] -> [128, (C//128)*n] device layout: row p, col t*n+j = m[t*128+p, j]."""
    n = m.shape[1]
    return np.ascontiguousarray(
        m.reshape(NE, 128, n).transpose(1, 0, 2).reshape(128, NE * n))


def _prep_inputs(x, attention_mask, pos_emb, attn_w, proj_w):
    import ml_dtypes

    x = np.asarray(x, dtype=np.float32)
    attention_mask = np.asarray(attention_mask)
    pos_emb = np.asarray(pos_emb, dtype=np.float32)
    attn_w = np.asarray(attn_w, dtype=np.float32)
    proj_w = np.asarray(proj_w, dtype=np.float32)

    x2 = x.reshape(B * W, CC, C)
    lens = attention_mask.sum(axis=2).reshape(B * W).astype(np.int64)
    lens = np.clip(lens, 1, CC)
    order = np.argsort(-lens, kind="stable")
    Lpos = lens[order[::NCORES]]                  # per-position max length
    rounds = _make_rounds(Lpos)
    TP = sum(w * l for w, l in rounds)

    wq = attn_w[:, :C]
    wk = attn_w[:, C:2 * C]
    wv = attn_w[:, 2 * C:]

    def wdev(wm):  # [C, C] -> [NF, 128, C] with [i, p, t*128+f] = wm[t*128+p, i*128+f]
        return np.ascontiguousarray(
            wm.reshape(NE, 128, NF, 128).transpose(2, 1, 0, 3).reshape(NF, 128, C))

    wq_d = wdev(wq).astype(ml_dtypes.bfloat16)
    wk_d = wdev(wk)
    wv_d = wdev(wv)
    wp_d = np.ascontiguousarray(proj_w.reshape(NF, 128, C))

    e16 = np.kron(np.eye(16, dtype=np.float32), np.ones((1, 64), np.float32))
    hw2 = np.zeros((128, 32), np.float32)
    hw2[np.arange(128), np.arange(128) // 64 + 14] = 1.0
    one16 = np.ones((1, 16), ml_dtypes.bfloat16)

    in_maps = []
    perms = []
    for core in range(NCORES):
        perm = order[np.arange(WPC) * NCORES + core]  # global word ids, desc len
        perms.append(perm)
        lc = lens[perm]
        xc = x2[perm]                                 # [128, 32, C]
        segs = []
        psegs = []
        w0 = 0
        for (Wr, Lr) in rounds:
            Lx = min(Lr, CC)
            xs = xc[w0:w0 + Wr, :Lx, :]               # [Wr, Lx, C]
            if Lx < Lr:                               # only if Lr bumped > CC
                xs = np.concatenate(
                    [xs, np.zeros((Wr, Lr - Lx, C), np.float32)], axis=1)
            segs.append(xs.reshape(Wr * Lr, C))
            mk = (np.arange(Lr)[None, :] < lc[w0:w0 + Wr][:, None])
            psegs.append(np.where(mk, 0.0, -1e30).astype(ml_dtypes.bfloat16).reshape(-1))
            w0 += Wr
        xp = np.concatenate(segs, axis=0)             # [TP, C]
        xT_dev = np.ascontiguousarray(xp.T.reshape(NE, 128, TP))
        xl = xc[np.arange(WPC), lc - 1]               # [128, C]
        xlT_dev = _tile_lhsT(np.ascontiguousarray(xl.T)).astype(
            ml_dtypes.bfloat16)
        posw = pos_emb[perm % W]                      # [128, C]
        posT_dev = _tile_lhsT(np.ascontiguousarray(posw.T))
        pen = np.concatenate(psegs).reshape(1, TP)    # 0 valid, -1e30 padded
        in_maps.append({
            "xT": xT_dev, "xlT": xlT_dev,
            "wq_d": wq_d, "wk_d": wk_d, "wv_d": wv_d, "wp_d": wp_d,
            "posT": posT_dev, "pen_in": np.ascontiguousarray(pen),
            "one16_in": one16, "e16_in": e16, "hw2_in": hw2,
        })
    return in_maps, rounds, perms


def kernel(x, attention_mask, pos_emb, attn_w, proj_w):
    global LAST_EXEC_NS
    from concourse.bass_utils import run_bass_kernel_spmd

    in_maps, rounds, perms = _prep_inputs(
        x, attention_mask, pos_emb, attn_w, proj_w)
    if _cache.get("rounds") != rounds:
        _cache["nc"] = _build_nc(rounds)
        _cache["rounds"] = rounds
    nc = _cache["nc"]
    trace = os.environ.get("KBENCH_TRACE") == "1"
    res = run_bass_kernel_spmd(nc, in_maps, core_ids=list(range(NCORES)),
                               trace=trace)
    if trace:
        LAST_EXEC_NS = res.exec_time_ns
    _cache["last_res"] = res
    full = np.empty((B * W, C), dtype=np.float32)
    for c in range(NCORES):
        full[perms[c]] = np.asarray(res.results[c]["out"], dtype=np.float32)
    return np.ascontiguousarray(full.reshape(B, W, C))


# revision 10
# speedup vs baseline: 1.0829x; 1.0829x over previous
"""CharAttention TRN2 kernel v4: 8-core data-parallel, ragged-packed,
round-pipelined, stall-free scheduling.

Only the LAST valid character's attention output is consumed by the
reference, so per word we compute
  q      = x[last] @ Wq                 (one query per word)
  K, V   = x @ Wk, x @ Wv               (valid positions only)
  scores = q . K / sqrt(hd) + pen, pen = -1e30 at j >= len
  o      = softmax(scores) @ V
  out    = (o + pos) @ Wp

Ragged packing: words are globally sorted by length (desc) and dealt
round-robin to the 8 cores, so position k on every core has nearly the
same length. A DP picks "rounds" of W_r words padded to a common
per-round length L_r with W_r*L_r <= 512 tokens, cutting the K/V
projection work from 4096 tokens/core to ~2400.

Round-outer pipeline: per round, the K projection accumulates 16-head
scores directly in PSUM (one matmul per f-tile against a head-summing
0/1 matrix, plus one 1-row matmul adding the -1e30 padding penalty),
softmax runs on ACT/DVE while the V projections keep the PE busy (the
head-broadcast matmuls of softmax(p) are skewed 2 f-tiles behind their
V projection so the PE never waits on the softmax chain — any PE gap
>3.4us re-throttles the clock to 1.2GHz). x arrives per (e-tile,
round) segment; K/V/P weights stay resident in SBUF (P prefetched
mid-pipeline); the output projection interleaves per-block. All
matmuls are float32r (1 cycle/row at N>=256); Q is bf16 (f32r is 4x
penalized at N=128).
"""
import os
import numpy as np

B, W, CC, C = 4, 256, 32, 1024
H, HD = 16, 64
NCORES = 8
WPC = (B * W) // NCORES          # 128 words per core
NE = 8                           # e-tiles (contraction over C)
NF = 8                           # f-tiles per 1024-wide projection
RLAM = 48                        # per-round fixed-cost in token units
SKEW = 2                         # pspe lags its psv by this many f-tiles

_cache = {}
LAST_EXEC_NS = None


def _round_shape(Wr, Lk):
    """Effective (L, tokens, pe_cost) for a round of Wr words, max len Lk.

    L >= Lk, total token count even (f32r matmul ISA requires an even
    free size) and >= 256 where possible (f32r is 4x penalized below)."""
    L = max(int(Lk), 1)
    if Wr * L < 256:
        L = min(CC, max(L, -(-256 // Wr)))
    if (Wr * L) % 2 and L < CC:
        L += 1
    T = Wr * L
    pe = T if T >= 256 else 4 * T
    return L, T, pe


def _make_rounds(Lpos):
    """DP partition of the 128 desc-sorted positions into rounds minimizing
    PE cost (padded tokens, <256-wide f32r penalty, per-round overhead)."""
    INF = float("inf")
    f = [INF] * (WPC + 1)
    nxt = [0] * (WPC + 1)
    f[WPC] = 0.0
    for k in range(WPC - 1, -1, -1):
        Lk = max(int(Lpos[k]), 1)
        wmax = min(max(512 // Lk, 1), WPC - k)
        for j in range(k + 1, k + wmax + 1):
            _, T, pe = _round_shape(j - k, Lk)
            if T > 512:
                continue
            c = f[j] + pe + RLAM
            if c < f[k]:
                f[k] = c
                nxt[k] = j
    rounds = []
    k = 0
    while k < WPC:
        j = nxt[k]
        L, _, _ = _round_shape(j - k, Lpos[k])
        rounds.append((j - k, L))
        k = j
    return tuple(rounds)


def _build_nc(rounds):
    import concourse.mybir as mybir
    import concourse.tile as tile
    from concourse import bacc

    f32 = mybir.dt.float32
    f32r = mybir.dt.float32r
    bf16 = mybir.dt.bfloat16
    Exp = mybir.ActivationFunctionType.Exp
    AX = mybir.AxisListType.X

    TP = sum(w * l for w, l in rounds)
    NR = len(rounds)
    woff = np.cumsum([0] + [w for w, _ in rounds])      # word offsets
    toff = np.cumsum([0] + [w * l for w, l in rounds])  # token offsets

    nc = bacc.Bacc("TRN2", target_bir_lowering=False, num_devices=NCORES,
                   debug=False)

    xT = nc.declare_dram_parameter("xT", [NE, 128, TP], bf16, isOutput=False)
    xlT = nc.declare_dram_parameter("xlT", [128, C], bf16, isOutput=False)
    wq_d = nc.declare_dram_parameter("wq_d", [NF, 128, C], bf16, isOutput=False)
    wk_d = nc.declare_dram_parameter("wk_d", [NF, 128, C], bf16, isOutput=False)
    wv_d = nc.declare_dram_parameter("wv_d", [NF, 128, C], bf16, isOutput=False)
    wp_d = nc.declare_dram_parameter("wp_d", [NF, 128, C], f32r, isOutput=False)
    posT = nc.declare_dram_parameter("posT", [128, C], f32, isOutput=False)
    pen_in = nc.declare_dram_parameter("pen_in", [1, TP], bf16, isOutput=False)
    one16_in = nc.declare_dram_parameter("one16_in", [1, 16], bf16,
                                         isOutput=False)
    e16_in = nc.declare_dram_parameter("e16_in", [16, C], f32r, isOutput=False)
    hw2_in = nc.declare_dram_parameter("hw2_in", [128, 32], f32r, isOutput=False)
    out = nc.declare_dram_parameter("out", [128, C], f32, isOutput=True)

    with tile.TileContext(nc) as tc:
        with tc.tile_pool(name="xpool", bufs=2) as xpool, \
             tc.tile_pool(name="wres", bufs=1) as wres, \
             tc.tile_pool(name="wpool", bufs=2) as wpool, \
             tc.tile_pool(name="work", bufs=2) as work, \
             tc.tile_pool(name="small", bufs=1) as small, \
             tc.tile_pool(name="psA", bufs=5, space="PSUM") as psA, \
             tc.tile_pool(name="psS", bufs=1, space="PSUM") as psS:

            # ---- resident loads (Q inputs first so PE starts early) ----
            xlT_sb = small.tile([128, C], bf16)
            nc.sync.dma_start(xlT_sb[:], xlT[:])

            qT_sb = small.tile([128, C], f32)
            s_sb = small.tile([16, WPC], f32)
            oT_sb = small.tile([128, C], f32r)

            # ---- Q projection: qT[f, w] accumulated over e-tiles (bf16) ----
            for i in range(NF):
                wq_t = wpool.tile([128, C], bf16, tag="wq")
                nc.sync.dma_start(wq_t[:], wq_d[i])
                psq = psA.tile([128, 128], f32, tag="mm")
                for t in range(NE):
                    nc.tensor.matmul(
                        psq[:], wq_t[:, t * 128:(t + 1) * 128],
                        xlT_sb[:, t * 128:(t + 1) * 128],
                        start=(t == 0), stop=(t == NE - 1))
                nc.vector.tensor_copy(qT_sb[:, i * 128:(i + 1) * 128], psq[:])

            # x segments for round 0 (scalar queue, in parallel with weights)
            xsegs = {}
            for t in range(NE):
                xs = xpool.tile([128, rounds[0][0] * rounds[0][1]], bf16,
                                tag=f"x{t}", name=f"xs{t}_0")
                nc.scalar.dma_start(xs[:], xT[t][:, 0:int(toff[1])])
                xsegs[(t, 0)] = xs

            # K weights resident (needed all within round 0)
            wk_sb = []
            for i in range(NF):
                wk_t = wres.tile([128, C], bf16, name=f"wk{i}")
                nc.sync.dma_start(wk_t[:], wk_d[i])
                wk_sb.append(wk_t)

            hw2_sb = small.tile([128, 32], f32r)
            nc.sync.dma_start(hw2_sb[:], hw2_in[:])
            one16_sb = small.tile([1, 16], bf16)
            nc.sync.dma_start(one16_sb[:], one16_in[:])
            pen_sb = small.tile([1, TP], bf16)
            nc.sync.dma_start(pen_sb[:], pen_in[:])
            e16_sb = small.tile([16, C], f32r)
            nc.sync.dma_start(e16_sb[:], e16_in[:])

            # V weights resident (needed all within round 0's V phase)
            wv_sb = []
            for i in range(NF):
                wv_t = wres.tile([128, C], bf16, name=f"wv{i}")
                nc.sync.dma_start(wv_t[:], wv_d[i])
                wv_sb.append(wv_t)

            posT_sb = small.tile([128, C], f32)
            nc.sync.dma_start(posT_sb[:], posT[:])

            wp_sb = []

            # ---- round pipeline ----
            for r in range(NR):
                Wr, Lr = rounds[r]
                TRr = Wr * Lr
                tseg = slice(int(toff[r]), int(toff[r + 1]))

                # prefetch x segments for round r+1
                if r + 1 < NR:
                    Wn, Ln = rounds[r + 1]
                    for t in range(NE):
                        xs = xpool.tile([128, Wn * Ln], bf16, tag=f"x{t}",
                                        name=f"xs{t}_{r + 1}")
                        nc.scalar.dma_start(
                            xs[:], xT[t][:, int(toff[r + 1]):int(toff[r + 2])])
                        xsegs[(t, r + 1)] = xs

                # K projection, scores accumulated in PSUM over f-tiles;
                # a final 1-row matmul adds the -1e30 padding penalty
                pss = psS.tile([16, TRr], f32, tag="psS")
                for i in range(NF):
                    psk = psA.tile([128, TRr], f32, tag="mm")
                    for t in range(NE):
                        nc.tensor.matmul(
                            psk[:], wk_sb[i][:, t * 128:(t + 1) * 128],
                            xsegs[(t, r)][:], start=(t == 0),
                            stop=(t == NE - 1))
                    # prod[(2h,64d), (w,j)] = K^T * qT broadcast over j
                    prod = work.tile([128, TRr], f32r, tag="prod")
                    qv = qT_sb[:, i * 128 + int(woff[r]):
                               i * 128 + int(woff[r + 1])]
                    nc.vector.tensor_mul(
                        prod[:].rearrange("p (w j) -> p w j", j=Lr),
                        psk[:].rearrange("p (w j) -> p w j", j=Lr),
                        qv[:, :, None].broadcast_to([128, Wr, Lr]))
                    # scores[h, (w,j)] += Hsum_i.T @ prod  (sums over d)
                    nc.tensor.matmul(
                        pss[:], hw2_sb[:, 14 - 2 * i: 30 - 2 * i],
                        prod[:], start=(i == 0), stop=False)
                nc.tensor.matmul(pss[:], one16_sb[:], pen_sb[:, tseg],
                                 start=False, stop=True)

                # softmax over j straight out of PSUM (padding already -inf)
                probs = work.tile([16, TRr], f32r, tag="probs")
                sv = s_sb[:, int(woff[r]):int(woff[r + 1])]
                nc.scalar.activation(probs[:], pss[:], Exp,
                                     scale=1.0 / float(np.sqrt(HD)))
                nc.vector.reduce_sum(
                    sv, probs[:].rearrange("p (w j) -> p w j", j=Lr), axis=AX)
                nc.vector.reciprocal(sv, sv)
                nc.vector.tensor_mul(
                    probs[:].rearrange("p (w j) -> p w j", j=Lr),
                    probs[:].rearrange("p (w j) -> p w j", j=Lr),
                    sv[:, :, None].broadcast_to([16, Wr, Lr]))

                # V projection; the probs head-broadcast (pspe) for f-tile i
                # is emitted after psv(i+SKEW) so the PE never waits on the
                # softmax chain
                psv_q = []

                def v_tail(i2):
                    psv2 = psv_q[i2]
                    pspe = psA.tile([128, TRr], f32, tag="pe",
                                    name=f"pspe{i2}_{r}", bufs=2)
                    nc.tensor.matmul(
                        pspe[:], e16_sb[:, i2 * 128:(i2 + 1) * 128],
                        probs[:], start=True, stop=True)
                    pexp_sb = work.tile([128, TRr], f32, tag="pexp",
                                        name=f"pexp{i2}_{r}")
                    nc.scalar.copy(pexp_sb[:], pspe[:])
                    prodv = work.tile([128, TRr], f32, tag="prodv",
                                      name=f"prodv{i2}_{r}")
                    nc.vector.tensor_mul(prodv[:], psv2[:], pexp_sb[:])
                    with nc.allow_low_precision(
                            reason="f32r out is bit-identical to f32"):
                        nc.vector.reduce_sum(
                            oT_sb[:, i2 * 128 + int(woff[r]):
                                  i2 * 128 + int(woff[r + 1])],
                            prodv[:].rearrange("p (w j) -> p w j", j=Lr),
                            axis=AX)

                for i in range(NF):
                    psv = psA.tile([128, TRr], f32, tag="mm",
                                   name=f"psv{i}_{r}")
                    for t in range(NE):
                        nc.tensor.matmul(
                            psv[:], wv_sb[i][:, t * 128:(t + 1) * 128],
                            xsegs[(t, r)][:], start=(t == 0),
                            stop=(t == NE - 1))
                    psv_q.append(psv)
                    if i >= SKEW:
                        v_tail(i - SKEW)
                for i2 in range(NF - SKEW, NF):
                    v_tail(i2)

                # prefetch the output-projection weights mid-pipeline
                if r == 1:
                    for i in range(NF):
                        wp_t = wres.tile([128, C], f32r, name=f"wp{i}")
                        nc.sync.dma_start(wp_t[:], wp_d[i])
                        wp_sb.append(wp_t)

            # ---- output projection: out[w, g] = (oT + posT).T @ Wp ----
            for i in range(NF):
                blk = slice(i * 128, (i + 1) * 128)
                nc.vector.tensor_add(oT_sb[:, blk], oT_sb[:, blk],
                                     posT_sb[:, blk])
            out_sb = small.tile([128, C], f32)
            pso = [psA.tile([128, 512], f32, tag="pe", name=f"pso{ch}",
                            bufs=2) for ch in range(2)]
            for i in range(NF):
                for ch in range(2):
                    nc.tensor.matmul(
                        pso[ch][:], oT_sb[:, i * 128:(i + 1) * 128],
                        wp_sb[i][:, ch * 512:(ch + 1) * 512],
                        start=(i == 0), stop=(i == NF - 1))
            for ch in range(2):
                nc.vector.tensor_copy(out_sb[:, ch * 512:(ch + 1) * 512],
                                      pso[ch][:])
                nc.sync.dma_start(out[:, ch * 512:(ch + 1) * 512],
                                  out_sb[:, ch * 512:(ch + 1) * 512])

    nc.finalize()
    return nc


def _tile_lhsT(m):
    """[C, n] -> [128, (C//128)*n] device layout: row p, col t*n+j = m[t*128+p, j]."""
    n = m.shape[1]
    return np.ascontiguousarray(
        m.reshape(NE, 128, n).transpose(1, 0, 2).reshape(128, NE * n))


def _prep_inputs(x, attention_mask, pos_emb, attn_w, proj_w):
    import ml_dtypes

    x = np.asarray(x, dtype=np.float32)
    attention_mask = np.asarray(attention_mask)
    pos_emb = np.asarray(pos_emb, dtype=np.float32)
    attn_w = np.asarray(attn_w, dtype=np.float32)
    proj_w = np.asarray(proj_w, dtype=np.float32)

    x2 = x.reshape(B * W, CC, C)
    lens = attention_mask.sum(axis=2).reshape(B * W).astype(np.int64)
    lens = np.clip(lens, 1, CC)
    order = np.argsort(-lens, kind="stable")
    Lpos = lens[order[::NCORES]]                  # per-position max length
    rounds = _make_rounds(Lpos)
    TP = sum(w * l for w, l in rounds)

    wq = attn_w[:, :C]
    wk = attn_w[:, C:2 * C]
    wv = attn_w[:, 2 * C:]

    def wdev(wm):  # [C, C] -> [NF, 128, C] with [i, p, t*128+f] = wm[t*128+p, i*128+f]
        return np.ascontiguousarray(
            wm.reshape(NE, 128, NF, 128).transpose(2, 1, 0, 3).reshape(NF, 128, C))

    wq_d = wdev(wq).astype(ml_dtypes.bfloat16)
    wk_d = wdev(wk).astype(ml_dtypes.bfloat16)
    wv_d = wdev(wv).astype(ml_dtypes.bfloat16)
    wp_d = np.ascontiguousarray(proj_w.reshape(NF, 128, C))

    e16 = np.kron(np.eye(16, dtype=np.float32), np.ones((1, 64), np.float32))
    hw2 = np.zeros((128, 32), np.float32)
    hw2[np.arange(128), np.arange(128) // 64 + 14] = 1.0
    one16 = np.ones((1, 16), ml_dtypes.bfloat16)

    in_maps = []
    perms = []
    for core in range(NCORES):
        perm = order[np.arange(WPC) * NCORES + core]  # global word ids, desc len
        perms.append(perm)
        lc = lens[perm]
        xc = x2[perm]                                 # [128, 32, C]
        segs = []
        psegs = []
        w0 = 0
        for (Wr, Lr) in rounds:
            Lx = min(Lr, CC)
            xs = xc[w0:w0 + Wr, :Lx, :]               # [Wr, Lx, C]
            if Lx < Lr:                               # only if Lr bumped > CC
                xs = np.concatenate(
                    [xs, np.zeros((Wr, Lr - Lx, C), np.float32)], axis=1)
            segs.append(xs.reshape(Wr * Lr, C))
            mk = (np.arange(Lr)[None, :] < lc[w0:w0 + Wr][:, None])
            psegs.append(np.where(mk, 0.0, -1e30).astype(ml_dtypes.bfloat16).reshape(-1))
            w0 += Wr
        xp = np.concatenate(segs, axis=0)             # [TP, C]
        xT_dev = np.ascontiguousarray(
            xp.T.reshape(NE, 128, TP).astype(ml_dtypes.bfloat16))
        xl = xc[np.arange(WPC), lc - 1]               # [128, C]
        xlT_dev = _tile_lhsT(np.ascontiguousarray(xl.T)).astype(
            ml_dtypes.bfloat16)
        posw = pos_emb[perm % W]                      # [128, C]
        posT_dev = _tile_lhsT(np.ascontiguousarray(posw.T))
        pen = np.concatenate(psegs).reshape(1, TP)    # 0 valid, -1e30 padded
        in_maps.append({
            "xT": xT_dev, "xlT": xlT_dev,
            "wq_d": wq_d, "wk_d": wk_d, "wv_d": wv_d, "wp_d": wp_d,
            "posT": posT_dev, "pen_in": np.ascontiguousarray(pen),
            "one16_in": one16, "e16_in": e16, "hw2_in": hw2,
        })
    return in_maps, rounds, perms


def kernel(x, attention_mask, pos_emb, attn_w, proj_w):
    global LAST_EXEC_NS
    from concourse.bass_utils import run_bass_kernel_spmd

    in_maps, rounds, perms = _prep_inputs(
        x, attention_mask, pos_emb, attn_w, proj_w)
    if _cache.get("rounds") != rounds:
        _cache["nc"] = _build_nc(rounds)
        _cache["rounds"] = rounds
    nc = _cache["nc"]
    trace = os.environ.get("KBENCH_TRACE") == "1"
    res = run_bass_kernel_spmd(nc, in_maps, core_ids=list(range(NCORES)),
                               trace=trace)
    if trace:
        LAST_EXEC_NS = res.exec_time_ns
    _cache["last_res"] = res
    full = np.empty((B * W, C), dtype=np.float32)
    for c in range(NCORES):
        full[perms[c]] = np.asarray(res.results[c]["out"], dtype=np.float32)
    return np.ascontiguousarray(full.reshape(B, W, C))
